# revision 9
# baseline (speedup 1.0000x reference)
"""Causal self-attention on 8 Trainium2 NeuronCores.

Problem: B=2, T=2048, C=1024, 16 heads x 64 dim, fp32.

Sharding: tensor-parallel over heads x data-parallel over batch.
Each core owns one batch element (cores 0-3 -> b=0, 4-7 -> b=1) and a
group of 4 consecutive heads. Each core computes:
  - QKV projection for its 4 heads (producing qT/kT transposed, V natural)
  - causal attention for its 4 heads (scores kept transposed: ST[tk, tq])
  - partial output projection (its heads' rows of w_proj)
The host sums the 4 partial projections per batch and adds b_proj.

Device layouts (all per core, fp32 in DRAM, matmuls run as float32r):
  xT   [1024, 2048]  x[b] transposed (channels on partitions)
  wqk  [1024, 512]   cols: q(h0)|q(h1)|k(h0)|k(h1)|q(h2)|q(h3)|k(h2)|k(h3)
  wv   [1024, 256]   v cols of the 4 heads
  wo   [256, 1024]   w_proj rows of the 4 heads
  bqk  [4, 128]      rows: pair0-q, pair0-k, pair1-q, pair1-k biases
  bv   [256]         v bias of the 4 heads
  mask [128, 128]    mask[i,j] = 1 if i<=j else 0 (tk<=tq keep)
  out  [2048, 1024]  partial (pre-bias) output projection

Attention math per head (pair tiles hold 2 heads at partitions 0-63/64-127):
  qT/kT [64, T] from matmul(lhsT=w_cols, rhs=xT)       (K=C, N=T slabs)
  ST    [tk, tq] = matmul(lhsT=kT tile, rhs=qT slab)   (K=64)
  expST = exp(0.125 * ST) on ACT, diagonal blocks masked by multiply
  yT_ext[65, tq] = matmul(lhsT=[V|1] tile, rhs=expST)  accumulated over tk
  yT = yT_ext[0:64] * reciprocal(yT_ext[64])           (softmax denominator)
  out += matmul(lhsT=yT tiles, rhs=wo)                 (K=256)
Causality: tk tiles > tq slab are skipped entirely; diagonal tk tiles only
compute columns tq >= tile start (partial-N matmuls).
"""

import numpy as np

B, T, C = 2, 2048, 1024
NH, DH = 16, 64
NCORES = 8
HPC = 4  # heads per core
P = 128
CK = C // P  # 8 contraction tiles over channels
NT = T // P  # 16 token tiles
SLAB = 512
NSL = T // SLAB  # 4 tq slabs

_CACHE = {}


def _build_program():
    from contextlib import ExitStack

    import concourse.bacc as bacc
    import concourse.bass as bass
    import concourse.tile as tile
    from concourse import mybir

    f32 = mybir.dt.float32
    f32r = mybir.dt.float32r
    AF = mybir.ActivationFunctionType

    nc = bacc.Bacc(
        "TRN2", target_bir_lowering=False, debug=False, num_devices=NCORES
    )

    f32r_ = mybir.dt.float32r
    xT = nc.dram_tensor("xT", [C, T], f32r_, kind="ExternalInput").ap()
    wqk = nc.dram_tensor("wqk", [C, 4 * P], f32r_, kind="ExternalInput").ap()
    wv = nc.dram_tensor("wv", [C, HPC * DH], f32r_, kind="ExternalInput").ap()
    wo = nc.dram_tensor("wo", [HPC * DH, C], f32r_, kind="ExternalInput").ap()
    bqk = nc.dram_tensor("bqk", [4, P], f32, kind="ExternalInput").ap()
    bv = nc.dram_tensor("bv", [HPC * DH], f32, kind="ExternalInput").ap()
    mask = nc.dram_tensor("mask", [P, P], f32, kind="ExternalInput").ap()
    out = nc.dram_tensor("out", [T, C], f32, kind="ExternalOutput").ap()

    def r(ap):
        return ap.bitcast(f32r)

    with tile.TileContext(nc) as tc, ExitStack() as ctx:
        const = ctx.enter_context(tc.tile_pool(name="const", bufs=1))
        stp = ctx.enter_context(tc.tile_pool(name="stp", bufs=3, space="PSUM"))
        yp = ctx.enter_context(tc.tile_pool(name="yp", bufs=3, space="PSUM"))
        op = ctx.enter_context(tc.tile_pool(name="op", bufs=2, space="PSUM"))
        expp = ctx.enter_context(tc.tile_pool(name="expp", bufs=3))
        rbp = ctx.enter_context(tc.tile_pool(name="rbp", bufs=2))
        outp = ctx.enter_context(tc.tile_pool(name="outp", bufs=3))

        x_sb = const.tile([P, CK, T], f32, name="x_sb")
        wqk_sb = const.tile([P, CK, 4 * P], f32, name="wqk_sb")
        wv_sb = const.tile([P, CK, HPC * DH], f32, name="wv_sb")
        wo_sb = const.tile([P, 2, C], f32, name="wo_sb")
        bqk_sb = const.tile([P, 4], f32, name="bqk_sb")
        bv_sb = const.tile([P, HPC, DH], f32, name="bv_sb")
        mask_sb = const.tile([P, P], f32, name="mask_sb")
        v_sb = const.tile([P, NT, HPC, DH + 1], f32, name="v_sb")
        qT = [const.tile([P, T], f32, name=f"qT{p}") for p in range(2)]
        kT = [const.tile([P, T], f32, name=f"kT{p}") for p in range(2)]
        yT = [const.tile([P, T], f32, name=f"yT{p}") for p in range(2)]

        # --- loads ---
        for k in range(CK):
            nc.sync.dma_start(
                out=r(x_sb[:, k, :]),
                in_=xT.rearrange("(k p) t -> k p t", p=P)[k],
            )
        nc.sync.dma_start(out=r(wqk_sb[:]), in_=wqk.rearrange("(k p) n -> p k n", p=P))
        nc.sync.dma_start(out=r(wv_sb[:]), in_=wv.rearrange("(k p) n -> p k n", p=P))
        nc.sync.dma_start(out=r(wo_sb[:]), in_=wo.rearrange("(r p) n -> p r n", p=P))
        nc.sync.dma_start(out=bqk_sb[:], in_=bqk.rearrange("r p -> p r"))
        bv_bcast = bass.AP(
            tensor=bv.tensor,
            offset=bv.offset,
            ap=[[0, P], *bv.rearrange("(h d) -> h d", d=DH).ap],
        )
        nc.sync.dma_start(out=bv_sb[:], in_=bv_bcast)
        nc.sync.dma_start(out=mask_sb[:], in_=mask)
        nc.vector.memset(v_sb[:, :, :, DH : DH + 1], 1.0)
        # rewrite the ones column as f32r (walrus requires rounded producers)
        nc.vector.tensor_scalar_mul(
            out=r(v_sb[:, :, :, DH : DH + 1]),
            in0=v_sb[:, :, :, DH : DH + 1],
            scalar1=1.0,
        )

        # --- Phase B: QKV projection ---
        # qT/kT pairs: partitions 0-63 = head 2p, 64-127 = head 2p+1
        for p in range(2):
            for qk in range(2):  # 0 = q, 1 = k
                dst = qT[p] if qk == 0 else kT[p]
                blk = 2 * p + qk
                for s in range(NSL):
                    ps = stp.tile([P, SLAB], f32, name="ps_qkv", tag="stps")
                    for k in range(CK):
                        nc.tensor.matmul(
                            ps[:],
                            lhsT=r(wqk_sb[:, k, blk * P : (blk + 1) * P]),
                            rhs=r(x_sb[:, k, s * SLAB : (s + 1) * SLAB]),
                            start=(k == 0),
                            stop=(k == CK - 1),
                        )
                    nc.scalar.activation(
                        out=r(dst[:, s * SLAB : (s + 1) * SLAB]),
                        in_=ps[:],
                        func=AF.Identity,
                        bias=bqk_sb[:, blk : blk + 1],
                        scale=1.0,
                    )
        # V (natural layout, tokens on partitions), ones column at DH
        for t in range(NT):
            ps = stp.tile([P, SLAB], f32, name="ps_v", tag="stps")
            for k in range(CK):
                nc.tensor.matmul(
                    ps[:, : HPC * DH],
                    lhsT=r(x_sb[:, k, t * P : (t + 1) * P]),
                    rhs=r(wv_sb[:, k, :]),
                    start=(k == 0),
                    stop=(k == CK - 1),
                )
            nc.vector.tensor_add(
                out=r(v_sb[:, t, :, 0:DH]),
                in0=ps[:, : HPC * DH].rearrange("p (h d) -> p h d", d=DH),
                in1=bv_sb[:],
            )

        # --- Phase C: causal attention ---
        for p in range(2):
            for s in range(NSL):
                psy = [
                    yp.tile([P, SLAB], f32, name=f"psy{hp}", tag="psy")
                    for hp in range(2)
                ]
                ntk = 4 * s + 4  # tk tiles 0 .. 4s+3 (causal)
                for tk in range(ntk):
                    diag_r = tk - 4 * s  # >= 0 on diagonal tiles
                    off = diag_r * P if diag_r >= 0 else 0
                    for hp in range(2):
                        lh = 2 * p + hp  # local head index
                        pst = stp.tile([P, SLAB], f32, name="ps_st", tag="stps")
                        nc.tensor.matmul(
                            pst[:, off:],
                            lhsT=r(kT[p][hp * DH : (hp + 1) * DH, tk * P : (tk + 1) * P]),
                            rhs=r(qT[p][hp * DH : (hp + 1) * DH, s * SLAB + off : (s + 1) * SLAB]),
                            start=True,
                            stop=True,
                        )
                        ex = expp.tile([P, SLAB], f32, name="ex", tag="ex")
                        nc.scalar.activation(
                            out=r(ex[:, off:]),
                            in_=pst[:, off:],
                            func=AF.Exp,
                            scale=float(1.0 / np.sqrt(DH)),
                        )
                        if diag_r >= 0:
                            nc.vector.tensor_mul(
                                out=r(ex[:, off : off + P]),
                                in0=r(ex[:, off : off + P]),
                                in1=mask_sb[:],
                            )
                        nc.tensor.matmul(
                            psy[hp][0 : DH + 1, off:],
                            lhsT=r(v_sb[:, tk, lh, :]),
                            rhs=r(ex[:, off:]),
                            start=(tk == 0),
                            stop=(tk == ntk - 1),
                        )
                for hp in range(2):
                    rec = rbp.tile([1, SLAB], f32, name="rec", tag="rec")
                    nc.vector.reciprocal(out=rec[:], in_=psy[hp][DH : DH + 1, :])
                    rb = rbp.tile([DH, SLAB], f32, name="rb", tag="rb")
                    nc.gpsimd.partition_broadcast(out_ap=rb[:], in_ap=rec[:])
                    nc.vector.tensor_mul(
                        out=r(yT[p][hp * DH : (hp + 1) * DH, s * SLAB : (s + 1) * SLAB]),
                        in0=psy[hp][0:DH, :],
                        in1=rb[:],
                    )

        # --- Phase D: output projection (partial; host adds b_proj & reduces) ---
        for t in range(NT):
            for ns in range(2):
                pso = op.tile([P, SLAB], f32, name="pso", tag="pso")
                for p in range(2):
                    nc.tensor.matmul(
                        pso[:],
                        lhsT=r(yT[p][:, t * P : (t + 1) * P]),
                        rhs=r(wo_sb[:, p, ns * SLAB : (ns + 1) * SLAB]),
                        start=(p == 0),
                        stop=(p == 1),
                    )
                ob = outp.tile([P, SLAB], f32, name="ob", tag="ob")
                nc.vector.tensor_copy(out=ob[:], in_=pso[:])
                nc.sync.dma_start(
                    out=out[t * P : (t + 1) * P, ns * SLAB : (ns + 1) * SLAB],
                    in_=ob[:],
                )

    nc.compile()
    return nc


def get_program():
    if "nc" not in _CACHE:
        _CACHE["nc"] = _build_program()
    return _CACHE["nc"]


def make_core_inputs(x, w_attn, b_attn, w_proj, core):
    """Host-side shard preparation for one core."""
    b = core // 4
    g = core % 4
    heads = [4 * g + i for i in range(HPC)]

    xT = np.ascontiguousarray(np.asarray(x[b], np.float32).T)

    def qcols(h):
        return w_attn[:, h * DH : (h + 1) * DH]

    def kcols(h):
        return w_attn[:, C + h * DH : C + (h + 1) * DH]

    def vcols(h):
        return w_attn[:, 2 * C + h * DH : 2 * C + (h + 1) * DH]

    h0, h1, h2, h3 = heads
    wqk = np.ascontiguousarray(
        np.concatenate(
            [qcols(h0), qcols(h1), kcols(h0), kcols(h1),
             qcols(h2), qcols(h3), kcols(h2), kcols(h3)],
            axis=1,
        ).astype(np.float32)
    )
    wv = np.ascontiguousarray(
        np.concatenate([vcols(h) for h in heads], axis=1).astype(np.float32)
    )
    bqk = np.stack(
        [
            np.concatenate([b_attn[h0 * DH : (h0 + 1) * DH], b_attn[h1 * DH : (h1 + 1) * DH]]),
            np.concatenate([b_attn[C + h0 * DH : C + (h0 + 1) * DH], b_attn[C + h1 * DH : C + (h1 + 1) * DH]]),
            np.concatenate([b_attn[h2 * DH : (h2 + 1) * DH], b_attn[h3 * DH : (h3 + 1) * DH]]),
            np.concatenate([b_attn[C + h2 * DH : C + (h2 + 1) * DH], b_attn[C + h3 * DH : C + (h3 + 1) * DH]]),
        ]
    ).astype(np.float32)
    bv = np.concatenate(
        [b_attn[2 * C + h * DH : 2 * C + (h + 1) * DH] for h in heads]
    ).astype(np.float32)
    wo = np.ascontiguousarray(
        w_proj[heads[0] * DH : (heads[-1] + 1) * DH, :].astype(np.float32)
    )
    mask = np.triu(np.ones((P, P), np.float32))
    return {
        "xT": xT,
        "wqk": wqk,
        "wv": wv,
        "wo": wo,
        "bqk": np.ascontiguousarray(bqk),
        "bv": np.ascontiguousarray(bv),
        "mask": mask,
    }


def kernel(x, w_attn, b_attn, w_proj, b_proj):
    from concourse.bass_utils import run_bass_kernel_spmd

    x = np.asarray(x, np.float32)
    w_attn = np.asarray(w_attn, np.float32)
    b_attn = np.asarray(b_attn, np.float32)
    w_proj = np.asarray(w_proj, np.float32)
    b_proj = np.asarray(b_proj, np.float32)

    nc = get_program()
    in_maps = [
        make_core_inputs(x, w_attn, b_attn, w_proj, core) for core in range(NCORES)
    ]
    res = run_bass_kernel_spmd(nc, in_maps, core_ids=list(range(NCORES)))
    outs = [m["out"] for m in res.results]

    y = np.empty((B, T, C), np.float32)
    for b in range(B):
        y[b] = outs[4 * b] + outs[4 * b + 1] + outs[4 * b + 2] + outs[4 * b + 3]
        y[b] += b_proj[None, :]
    return y


# revision 15
# speedup vs baseline: 1.3840x; 1.3840x over previous
"""Causal self-attention on 8 Trainium2 NeuronCores.

Problem: B=2, T=2048, C=1024, 16 heads x 64 dim, fp32.

Sharding: tensor-parallel over heads x data-parallel over batch.
Each core owns one batch element (cores 0-3 -> b=0, 4-7 -> b=1) and a
group of 4 consecutive heads. Each core computes:
  - QKV projection for its 4 heads (producing qT/kT transposed, V natural)
  - causal attention for its 4 heads (scores kept transposed: ST[tk, tq])
  - partial output projection (its heads' rows of w_proj)
The host sums the 4 partial projections per batch and adds b_proj.

Device layouts (all per core, fp32 in DRAM, matmuls run as float32r):
  xT   [1024, 2048]  x[b] transposed (channels on partitions)
  wqk  [1024, 512]   cols: q(h0)|q(h1)|k(h0)|k(h1)|q(h2)|q(h3)|k(h2)|k(h3)
  wv   [1024, 256]   v cols of the 4 heads
  wo   [256, 1024]   w_proj rows of the 4 heads
  bqk  [4, 128]      rows: pair0-q, pair0-k, pair1-q, pair1-k biases
  bv   [256]         v bias of the 4 heads
  mask [128, 128]    mask[i,j] = 1 if i<=j else 0 (tk<=tq keep)
  out  [2048, 1024]  partial (pre-bias) output projection

Attention math per head (pair tiles hold 2 heads at partitions 0-63/64-127):
  qT/kT [64, T] from matmul(lhsT=w_cols, rhs=xT)       (K=C, N=T slabs)
  ST    [tk, tq] = matmul(lhsT=kT tile, rhs=qT slab)   (K=64)
  expST = exp(0.125 * ST) on ACT, diagonal blocks masked by multiply
  yT_ext[65, tq] = matmul(lhsT=[V|1] tile, rhs=expST)  accumulated over tk
  yT = yT_ext[0:64] * reciprocal(yT_ext[64])           (softmax denominator)
  out += matmul(lhsT=yT tiles, rhs=wo)                 (K=256)
Causality: tk tiles > tq slab are skipped entirely; diagonal tk tiles only
compute columns tq >= tile start (partial-N matmuls).
"""

import numpy as np

B, T, C = 2, 2048, 1024
NH, DH = 16, 64
NCORES = 8
HPC = 4  # heads per core
P = 128
CK = C // P  # 8 contraction tiles over channels
NT = T // P  # 16 token tiles
SLAB = 512
NSL = T // SLAB  # 4 tq slabs

_CACHE = {}


def _build_program():
    from contextlib import ExitStack

    import concourse.bacc as bacc
    import concourse.bass as bass
    import concourse.tile as tile
    from concourse import mybir

    f32 = mybir.dt.float32
    f32r = mybir.dt.float32r
    AF = mybir.ActivationFunctionType

    nc = bacc.Bacc(
        "TRN2", target_bir_lowering=False, debug=False, num_devices=NCORES
    )

    f32r_ = mybir.dt.float32r
    xT = nc.dram_tensor("xT", [C, T], f32r_, kind="ExternalInput").ap()
    wqk = nc.dram_tensor("wqk", [C, 4 * P], f32r_, kind="ExternalInput").ap()
    wv = nc.dram_tensor("wv", [C, HPC * DH], f32r_, kind="ExternalInput").ap()
    wo = nc.dram_tensor("wo", [HPC * DH, C], f32r_, kind="ExternalInput").ap()
    bqk = nc.dram_tensor("bqk", [4, P], f32, kind="ExternalInput").ap()
    bv = nc.dram_tensor("bv", [HPC * DH], f32, kind="ExternalInput").ap()
    mask = nc.dram_tensor("mask", [P, P], f32, kind="ExternalInput").ap()
    out = nc.dram_tensor("out", [T, C], f32, kind="ExternalOutput").ap()

    def r(ap):
        return ap.bitcast(f32r)

    with tile.TileContext(nc) as tc, ExitStack() as ctx:
        const = ctx.enter_context(tc.tile_pool(name="const", bufs=1))
        # PSUM budget: 3 x [128,1024] (6 banks) shared by QKV/ST/outproj
        # + 2 x [128,512] (2 banks) for the PV accumulators = 8 banks.
        stp = ctx.enter_context(tc.tile_pool(name="stp", bufs=3, space="PSUM"))
        yp = ctx.enter_context(tc.tile_pool(name="yp", bufs=2, space="PSUM"))
        expp = ctx.enter_context(tc.tile_pool(name="expp", bufs=3))
        rbp = ctx.enter_context(tc.tile_pool(name="rbp", bufs=2))
        outp = ctx.enter_context(tc.tile_pool(name="outp", bufs=3))

        x_sb = const.tile([P, CK, T], f32, name="x_sb")
        wqk_sb = const.tile([P, CK, 4 * P], f32, name="wqk_sb")
        wv_sb = const.tile([P, CK, HPC * DH], f32, name="wv_sb")
        wo_sb = const.tile([P, 2, C], f32, name="wo_sb")
        bqk_sb = const.tile([P, 4], f32, name="bqk_sb")
        bv_sb = const.tile([P, HPC, DH], f32, name="bv_sb")
        mask_sb = const.tile([P, P], f32, name="mask_sb")
        v_sb = const.tile([P, NT, HPC, DH + 1], f32, name="v_sb")
        qT = [const.tile([P, T], f32, name=f"qT{p}") for p in range(2)]
        kT = [const.tile([P, T], f32, name=f"kT{p}") for p in range(2)]
        yT = [const.tile([P, T], f32, name=f"yT{p}") for p in range(2)]

        # --- loads ---
        for k in range(CK):
            nc.sync.dma_start(
                out=r(x_sb[:, k, :]),
                in_=xT.rearrange("(k p) t -> k p t", p=P)[k],
            )
        nc.sync.dma_start(out=r(wqk_sb[:]), in_=wqk.rearrange("(k p) n -> p k n", p=P))
        nc.sync.dma_start(out=r(wv_sb[:]), in_=wv.rearrange("(k p) n -> p k n", p=P))
        nc.sync.dma_start(out=r(wo_sb[:]), in_=wo.rearrange("(r p) n -> p r n", p=P))
        nc.sync.dma_start(out=bqk_sb[:], in_=bqk.rearrange("r p -> p r"))
        bv_bcast = bass.AP(
            tensor=bv.tensor,
            offset=bv.offset,
            ap=[[0, P], *bv.rearrange("(h d) -> h d", d=DH).ap],
        )
        nc.sync.dma_start(out=bv_sb[:], in_=bv_bcast)
        nc.sync.dma_start(out=mask_sb[:], in_=mask)
        nc.vector.memset(v_sb[:, :, :, DH : DH + 1], 1.0)
        # rewrite the ones column as f32r (walrus requires rounded producers)
        nc.vector.tensor_scalar_mul(
            out=r(v_sb[:, :, :, DH : DH + 1]),
            in0=v_sb[:, :, :, DH : DH + 1],
            scalar1=1.0,
        )

        # --- Phase B: QKV projection ---
        # qT/kT pairs: partitions 0-63 = head 2p, 64-127 = head 2p+1
        for p in range(2):
            for qk in range(2):  # 0 = q, 1 = k
                dst = qT[p] if qk == 0 else kT[p]
                blk = 2 * p + qk
                for s in range(NSL):
                    ps = stp.tile([P, 2 * SLAB], f32, name="ps_qkv", tag="big")[
                        :, :SLAB
                    ]
                    for k in range(CK):
                        nc.tensor.matmul(
                            ps[:],
                            lhsT=r(wqk_sb[:, k, blk * P : (blk + 1) * P]),
                            rhs=r(x_sb[:, k, s * SLAB : (s + 1) * SLAB]),
                            start=(k == 0),
                            stop=(k == CK - 1),
                        )
                    nc.scalar.activation(
                        out=r(dst[:, s * SLAB : (s + 1) * SLAB]),
                        in_=ps[:],
                        func=AF.Identity,
                        bias=bqk_sb[:, blk : blk + 1],
                        scale=1.0,
                    )
        # V (natural layout, tokens on partitions), ones column at DH
        for t in range(NT):
            ps = stp.tile([P, 2 * SLAB], f32, name="ps_v", tag="big")[:, :SLAB]
            for k in range(CK):
                nc.tensor.matmul(
                    ps[:, : HPC * DH],
                    lhsT=r(x_sb[:, k, t * P : (t + 1) * P]),
                    rhs=r(wv_sb[:, k, :]),
                    start=(k == 0),
                    stop=(k == CK - 1),
                )
            nc.vector.tensor_add(
                out=r(v_sb[:, t, :, 0:DH]),
                in0=ps[:, : HPC * DH].rearrange("p (h d) -> p h d", d=DH),
                in1=bv_sb[:],
            )

        # --- Phase C: causal attention ---
        # Both heads of a pair share one 2-bank ST psum tile [128, 1024]
        # (cols 0-511 head even, 512-1023 head odd) so exp is a single ACT
        # instruction per tk. ST matmuls are issued 2 tk ahead of the PV
        # matmuls so the PE never stalls waiting for ACT's exp.
        for p in range(2):
            for s in range(NSL):
                psy = [
                    yp.tile([P, SLAB], f32, name=f"psy{hp}", tag="psy")
                    for hp in range(2)
                ]
                ntk = 4 * s + 4  # tk tiles 0 .. 4s+3 (causal)

                def off_of(tk):
                    diag_r = tk - 4 * s
                    return diag_r * P if diag_r >= 0 else 0

                def st_pair(tk):
                    off = off_of(tk)
                    pp = stp.tile([P, 2 * SLAB], f32, name="pp", tag="big")
                    for hp in range(2):
                        nc.tensor.matmul(
                            pp[:, hp * SLAB + off : (hp + 1) * SLAB],
                            lhsT=r(kT[p][hp * DH : (hp + 1) * DH, tk * P : (tk + 1) * P]),
                            rhs=r(qT[p][hp * DH : (hp + 1) * DH, s * SLAB + off : (s + 1) * SLAB]),
                            start=True,
                            stop=True,
                        )
                    return pp

                pend = {0: st_pair(0)}
                if ntk > 1:
                    pend[1] = st_pair(1)
                for tk in range(ntk):
                    off = off_of(tk)
                    pp = pend.pop(tk)
                    ex = expp.tile([P, 2 * SLAB], f32, name="ex", tag="ex")
                    ppv = pp[:].rearrange("q (h n) -> q h n", h=2)[:, :, off:]
                    exv = ex[:].rearrange("q (h n) -> q h n", h=2)[:, :, off:]
                    nc.scalar.activation(
                        out=r(exv),
                        in_=ppv,
                        func=AF.Exp,
                        scale=float(1.0 / np.sqrt(DH)),
                    )
                    if tk - 4 * s >= 0:
                        for hp in range(2):
                            nc.vector.tensor_mul(
                                out=r(ex[:, hp * SLAB + off : hp * SLAB + off + P]),
                                in0=r(ex[:, hp * SLAB + off : hp * SLAB + off + P]),
                                in1=mask_sb[:],
                            )
                    if tk + 2 < ntk:
                        pend[tk + 2] = st_pair(tk + 2)
                    for hp in range(2):
                        nc.tensor.matmul(
                            psy[hp][0 : DH + 1, off:],
                            lhsT=r(v_sb[:, tk, 2 * p + hp, :]),
                            rhs=r(ex[:, hp * SLAB + off : (hp + 1) * SLAB]),
                            start=(tk == 0),
                            stop=(tk == ntk - 1),
                        )
                for hp in range(2):
                    sm = rbp.tile([1, SLAB], f32, name="sm", tag="sm")
                    nc.vector.tensor_copy(out=sm[:], in_=psy[hp][DH : DH + 1, :])
                    rec = rbp.tile([1, SLAB], f32, name="rec", tag="rec")
                    nc.vector.reciprocal_approx_fast(out=rec[:], in_=sm[:])
                    rb = rbp.tile([DH, SLAB], f32, name="rb", tag="rb")
                    nc.gpsimd.partition_broadcast(out_ap=rb[:], in_ap=rec[:])
                    nc.vector.tensor_mul(
                        out=r(yT[p][hp * DH : (hp + 1) * DH, s * SLAB : (s + 1) * SLAB]),
                        in0=psy[hp][0:DH, :],
                        in1=rb[:],
                    )

        # --- Phase D: output projection (partial; host adds b_proj & reduces) ---
        for t in range(NT):
            pso = stp.tile([P, 2 * SLAB], f32, name="pso", tag="big")
            for ns in range(2):
                for p in range(2):
                    nc.tensor.matmul(
                        pso[:, ns * SLAB : (ns + 1) * SLAB],
                        lhsT=r(yT[p][:, t * P : (t + 1) * P]),
                        rhs=r(wo_sb[:, p, ns * SLAB : (ns + 1) * SLAB]),
                        start=(p == 0),
                        stop=(p == 1),
                    )
            ob = outp.tile([P, 2 * SLAB], f32, name="ob", tag="ob")
            nc.vector.tensor_copy(out=ob[:], in_=pso[:])
            nc.sync.dma_start(out=out[t * P : (t + 1) * P, :], in_=ob[:])

    nc.compile()
    return nc


def get_program():
    if "nc" not in _CACHE:
        _CACHE["nc"] = _build_program()
    return _CACHE["nc"]


def make_core_inputs(x, w_attn, b_attn, w_proj, core):
    """Host-side shard preparation for one core."""
    b = core // 4
    g = core % 4
    heads = [4 * g + i for i in range(HPC)]

    xT = np.ascontiguousarray(np.asarray(x[b], np.float32).T)

    def qcols(h):
        return w_attn[:, h * DH : (h + 1) * DH]

    def kcols(h):
        return w_attn[:, C + h * DH : C + (h + 1) * DH]

    def vcols(h):
        return w_attn[:, 2 * C + h * DH : 2 * C + (h + 1) * DH]

    h0, h1, h2, h3 = heads
    wqk = np.ascontiguousarray(
        np.concatenate(
            [qcols(h0), qcols(h1), kcols(h0), kcols(h1),
             qcols(h2), qcols(h3), kcols(h2), kcols(h3)],
            axis=1,
        ).astype(np.float32)
    )
    wv = np.ascontiguousarray(
        np.concatenate([vcols(h) for h in heads], axis=1).astype(np.float32)
    )
    bqk = np.stack(
        [
            np.concatenate([b_attn[h0 * DH : (h0 + 1) * DH], b_attn[h1 * DH : (h1 + 1) * DH]]),
            np.concatenate([b_attn[C + h0 * DH : C + (h0 + 1) * DH], b_attn[C + h1 * DH : C + (h1 + 1) * DH]]),
            np.concatenate([b_attn[h2 * DH : (h2 + 1) * DH], b_attn[h3 * DH : (h3 + 1) * DH]]),
            np.concatenate([b_attn[C + h2 * DH : C + (h2 + 1) * DH], b_attn[C + h3 * DH : C + (h3 + 1) * DH]]),
        ]
    ).astype(np.float32)
    bv = np.concatenate(
        [b_attn[2 * C + h * DH : 2 * C + (h + 1) * DH] for h in heads]
    ).astype(np.float32)
    wo = np.ascontiguousarray(
        w_proj[heads[0] * DH : (heads[-1] + 1) * DH, :].astype(np.float32)
    )
    mask = np.triu(np.ones((P, P), np.float32))
    return {
        "xT": xT,
        "wqk": wqk,
        "wv": wv,
        "wo": wo,
        "bqk": np.ascontiguousarray(bqk),
        "bv": np.ascontiguousarray(bv),
        "mask": mask,
    }


def kernel(x, w_attn, b_attn, w_proj, b_proj):
    from concourse.bass_utils import run_bass_kernel_spmd

    x = np.asarray(x, np.float32)
    w_attn = np.asarray(w_attn, np.float32)
    b_attn = np.asarray(b_attn, np.float32)
    w_proj = np.asarray(w_proj, np.float32)
    b_proj = np.asarray(b_proj, np.float32)

    nc = get_program()
    in_maps = [
        make_core_inputs(x, w_attn, b_attn, w_proj, core) for core in range(NCORES)
    ]
    res = run_bass_kernel_spmd(nc, in_maps, core_ids=list(range(NCORES)))
    outs = [m["out"] for m in res.results]

    y = np.empty((B, T, C), np.float32)
    for b in range(B):
        y[b] = outs[4 * b] + outs[4 * b + 1] + outs[4 * b + 2] + outs[4 * b + 3]
        y[b] += b_proj[None, :]
    return y


# revision 18
# speedup vs baseline: 1.5436x; 1.1153x over previous
"""Causal self-attention on 8 Trainium2 NeuronCores.

Problem: B=2, T=2048, C=1024, 16 heads x 64 dim, fp32.

Sharding: tensor-parallel over heads x data-parallel over batch.
Each core owns one batch element (cores 0-3 -> b=0, 4-7 -> b=1) and a
group of 4 consecutive heads. Each core computes:
  - QKV projection for its 4 heads (producing qT/kT transposed, V natural)
  - causal attention for its 4 heads (scores kept transposed: ST[tk, tq])
  - partial output projection (its heads' rows of w_proj)
The host sums the 4 partial projections per batch and adds b_proj.

Device layouts (all per core, fp32 in DRAM, matmuls run as float32r):
  xT   [1024, 2048]  x[b] transposed (channels on partitions)
  wqk  [1024, 512]   cols: q(h0)|q(h1)|k(h0)|k(h1)|q(h2)|q(h3)|k(h2)|k(h3)
  wv   [1024, 256]   v cols of the 4 heads
  wo   [256, 1024]   w_proj rows of the 4 heads
  bqk  [4, 128]      rows: pair0-q, pair0-k, pair1-q, pair1-k biases
  bv   [256]         v bias of the 4 heads
  mask [128, 128]    mask[i,j] = 1 if i<=j else 0 (tk<=tq keep)
  out  [2048, 1024]  partial (pre-bias) output projection

Attention math per head (pair tiles hold 2 heads at partitions 0-63/64-127):
  qT/kT [64, T] from matmul(lhsT=w_cols, rhs=xT)       (K=C, N=T slabs)
  ST    [tk, tq] = matmul(lhsT=kT tile, rhs=qT slab)   (K=64)
  expST = exp(0.125 * ST) on ACT, diagonal blocks masked by multiply
  yT_ext[65, tq] = matmul(lhsT=[V|1] tile, rhs=expST)  accumulated over tk
  yT = yT_ext[0:64] * reciprocal(yT_ext[64])           (softmax denominator)
  out += matmul(lhsT=yT tiles, rhs=wo)                 (K=256)
Causality: tk tiles > tq slab are skipped entirely; diagonal tk tiles only
compute columns tq >= tile start (partial-N matmuls).
"""

import numpy as np

B, T, C = 2, 2048, 1024
NH, DH = 16, 64
NCORES = 8
HPC = 4  # heads per core
P = 128
CK = C // P  # 8 contraction tiles over channels
NT = T // P  # 16 token tiles
SLAB = 512
NSL = T // SLAB  # 4 tq slabs

_CACHE = {}


def _build_program():
    from contextlib import ExitStack

    import concourse.bacc as bacc
    import concourse.bass as bass
    import concourse.tile as tile
    from concourse import mybir

    f32 = mybir.dt.float32
    f32r = mybir.dt.float32r
    bf16 = mybir.dt.bfloat16
    AF = mybir.ActivationFunctionType

    nc = bacc.Bacc(
        "TRN2", target_bir_lowering=False, debug=False, num_devices=NCORES
    )

    f32r_ = mybir.dt.float32r
    xT = nc.dram_tensor("xT", [C, T], f32r_, kind="ExternalInput").ap()
    wqk = nc.dram_tensor("wqk", [C, 4 * P], f32r_, kind="ExternalInput").ap()
    wv = nc.dram_tensor("wv", [C, HPC * DH], f32r_, kind="ExternalInput").ap()
    wo = nc.dram_tensor("wo", [HPC * DH, C], f32r_, kind="ExternalInput").ap()
    bqk = nc.dram_tensor("bqk", [4, P], f32, kind="ExternalInput").ap()
    bv = nc.dram_tensor("bv", [HPC * DH], f32, kind="ExternalInput").ap()
    mask = nc.dram_tensor("mask", [P, P], bf16, kind="ExternalInput").ap()
    out = nc.dram_tensor("out", [T, C], f32, kind="ExternalOutput").ap()

    def r(ap):
        return ap.bitcast(f32r)

    with tile.TileContext(nc) as tc, ExitStack() as ctx:
        const = ctx.enter_context(tc.tile_pool(name="const", bufs=1))
        # PSUM budget: 3 x [128,1024] (6 banks) shared by QKV/ST/outproj
        # + 2 x [128,512] (2 banks) for the PV accumulators = 8 banks.
        stp = ctx.enter_context(tc.tile_pool(name="stp", bufs=3, space="PSUM"))
        yp = ctx.enter_context(tc.tile_pool(name="yp", bufs=2, space="PSUM"))
        expp = ctx.enter_context(tc.tile_pool(name="expp", bufs=4))
        rbp = ctx.enter_context(tc.tile_pool(name="rbp", bufs=2))
        outp = ctx.enter_context(tc.tile_pool(name="outp", bufs=3))

        x_ch = [
            const.tile([P, CK, SLAB], f32, name=f"x_ch{c}") for c in range(NSL)
        ]
        wqk_sb = const.tile([P, CK, 4 * P], f32, name="wqk_sb")
        wv_sb = const.tile([P, CK, HPC * DH], f32, name="wv_sb")
        wo_sb = const.tile([P, 2, C], f32, name="wo_sb")
        bqk_sb = const.tile([P, 4], f32, name="bqk_sb")
        bv_sb = const.tile([P, HPC, DH], f32, name="bv_sb")
        mask_sb = const.tile([P, P], bf16, name="mask_sb")
        v_sb = const.tile([P, NT, HPC, DH + 1], bf16, name="v_sb")
        qT = [const.tile([P, T], bf16, name=f"qT{p}") for p in range(2)]
        kT = [const.tile([P, T], bf16, name=f"kT{p}") for p in range(2)]
        yT = [const.tile([P, T], f32, name=f"yT{p}") for p in range(2)]

        # --- loads (weights first, then x chunk-major so compute starts early) ---
        nc.sync.dma_start(out=r(wqk_sb[:]), in_=wqk.rearrange("(k p) n -> p k n", p=P))
        xTv = xT.rearrange("(k p) t -> k p t", p=P)
        for c in range(NSL):
            for k in range(CK):
                nc.sync.dma_start(
                    out=r(x_ch[c][:, k, :]),
                    in_=xTv[k][:, c * SLAB : (c + 1) * SLAB],
                )
        nc.sync.dma_start(out=r(wv_sb[:]), in_=wv.rearrange("(k p) n -> p k n", p=P))
        nc.sync.dma_start(out=r(wo_sb[:]), in_=wo.rearrange("(r p) n -> p r n", p=P))
        nc.sync.dma_start(out=bqk_sb[:], in_=bqk.rearrange("r p -> p r"))
        bv_bcast = bass.AP(
            tensor=bv.tensor,
            offset=bv.offset,
            ap=[[0, P], *bv.rearrange("(h d) -> h d", d=DH).ap],
        )
        nc.sync.dma_start(out=bv_sb[:], in_=bv_bcast)
        nc.sync.dma_start(out=mask_sb[:], in_=mask)
        nc.vector.memset(v_sb[:, :, :, DH : DH + 1], 1.0)

        # --- Phase B: QKV projection, s-outer so slab 0 starts after chunk 0 ---
        # qT/kT pairs: partitions 0-63 = head 2p, 64-127 = head 2p+1
        for s in range(NSL):
            for blk in range(4):  # (pair, q/k) column blocks
                p, qk = divmod(blk, 2)
                dst = qT[p] if qk == 0 else kT[p]
                ps = stp.tile([P, 2 * SLAB], f32, name="ps_qkv", tag="big")[
                    :, :SLAB
                ]
                for k in range(CK):
                    nc.tensor.matmul(
                        ps[:],
                        lhsT=r(wqk_sb[:, k, blk * P : (blk + 1) * P]),
                        rhs=r(x_ch[s][:, k, :]),
                        start=(k == 0),
                        stop=(k == CK - 1),
                    )
                nc.scalar.activation(
                    out=dst[:, s * SLAB : (s + 1) * SLAB],
                    in_=ps[:],
                    func=AF.Identity,
                    bias=bqk_sb[:, blk : blk + 1],
                    scale=1.0,
                )
            # V tiles of this chunk (natural layout), ones column at DH
            for tt in range(4):
                t = 4 * s + tt
                ps = stp.tile([P, 2 * SLAB], f32, name="ps_v", tag="big")[:, :SLAB]
                for k in range(CK):
                    nc.tensor.matmul(
                        ps[:, : HPC * DH],
                        lhsT=r(x_ch[s][:, k, tt * P : (tt + 1) * P]),
                        rhs=r(wv_sb[:, k, :]),
                        start=(k == 0),
                        stop=(k == CK - 1),
                    )
                nc.vector.tensor_add(
                    out=v_sb[:, t, :, 0:DH],
                    in0=ps[:, : HPC * DH].rearrange("p (h d) -> p h d", d=DH),
                    in1=bv_sb[:],
                )

        # --- Phase C: causal attention ---
        # Both heads of a pair share one 2-bank ST psum tile [128, 1024]
        # (cols 0-511 head even, 512-1023 head odd) so exp is a single ACT
        # instruction per tk. ST matmuls are issued 2 tk ahead of the PV
        # matmuls so the PE never stalls waiting for ACT's exp.
        for p in range(2):
            for s in range(NSL):
                psy = [
                    yp.tile([P, SLAB], f32, name=f"psy{hp}", tag="psy")
                    for hp in range(2)
                ]
                ntk = 4 * s + 4  # tk tiles 0 .. 4s+3 (causal)

                def off_of(tk):
                    diag_r = tk - 4 * s
                    return diag_r * P if diag_r >= 0 else 0

                def st_pair(tk):
                    off = off_of(tk)
                    pp = stp.tile([P, 2 * SLAB], f32, name="pp", tag="big")
                    for hp in range(2):
                        nc.tensor.matmul(
                            pp[:, hp * SLAB + off : (hp + 1) * SLAB],
                            lhsT=kT[p][hp * DH : (hp + 1) * DH, tk * P : (tk + 1) * P],
                            rhs=qT[p][hp * DH : (hp + 1) * DH, s * SLAB + off : (s + 1) * SLAB],
                            start=True,
                            stop=True,
                        )
                    return pp

                def do_exp(tk):
                    off = off_of(tk)
                    pp = pend.pop(tk)
                    ex = expp.tile([P, 2 * SLAB], bf16, name="ex", tag="ex")
                    ppv = pp[:].rearrange("q (h n) -> q h n", h=2)[:, :, off:]
                    exv = ex[:].rearrange("q (h n) -> q h n", h=2)[:, :, off:]
                    nc.scalar.activation(
                        out=exv,
                        in_=ppv,
                        func=AF.Exp,
                        scale=float(1.0 / np.sqrt(DH)),
                    )
                    if tk - 4 * s >= 0:
                        for hp in range(2):
                            nc.vector.tensor_mul(
                                out=ex[:, hp * SLAB + off : hp * SLAB + off + P],
                                in0=ex[:, hp * SLAB + off : hp * SLAB + off + P],
                                in1=mask_sb[:],
                            )
                    return ex

                pend = {0: st_pair(0)}
                if ntk > 1:
                    pend[1] = st_pair(1)
                exd = {0: do_exp(0)}
                for tk in range(ntk):
                    off = off_of(tk)
                    if tk + 2 < ntk:
                        pend[tk + 2] = st_pair(tk + 2)
                    if tk + 1 < ntk:
                        exd[tk + 1] = do_exp(tk + 1)
                    ex = exd.pop(tk)
                    for hp in range(2):
                        nc.tensor.matmul(
                            psy[hp][0 : DH + 1, off:],
                            lhsT=v_sb[:, tk, 2 * p + hp, :],
                            rhs=ex[:, hp * SLAB + off : (hp + 1) * SLAB],
                            start=(tk == 0),
                            stop=(tk == ntk - 1),
                        )
                for hp in range(2):
                    sm = rbp.tile([1, SLAB], f32, name="sm", tag="sm")
                    nc.vector.tensor_copy(out=sm[:], in_=psy[hp][DH : DH + 1, :])
                    rec = rbp.tile([1, SLAB], f32, name="rec", tag="rec")
                    nc.vector.reciprocal_approx_fast(out=rec[:], in_=sm[:])
                    rb = rbp.tile([DH, SLAB], f32, name="rb", tag="rb")
                    nc.gpsimd.partition_broadcast(out_ap=rb[:], in_ap=rec[:])
                    nc.vector.tensor_mul(
                        out=r(yT[p][hp * DH : (hp + 1) * DH, s * SLAB : (s + 1) * SLAB]),
                        in0=psy[hp][0:DH, :],
                        in1=rb[:],
                    )

        # --- Phase D: output projection (partial; host adds b_proj & reduces) ---
        for t in range(NT):
            pso = stp.tile([P, 2 * SLAB], f32, name="pso", tag="big")
            for ns in range(2):
                for p in range(2):
                    nc.tensor.matmul(
                        pso[:, ns * SLAB : (ns + 1) * SLAB],
                        lhsT=r(yT[p][:, t * P : (t + 1) * P]),
                        rhs=r(wo_sb[:, p, ns * SLAB : (ns + 1) * SLAB]),
                        start=(p == 0),
                        stop=(p == 1),
                    )
            ob = outp.tile([P, 2 * SLAB], f32, name="ob", tag="ob")
            nc.vector.tensor_copy(out=ob[:], in_=pso[:])
            nc.sync.dma_start(out=out[t * P : (t + 1) * P, :], in_=ob[:])

    nc.compile()
    return nc


def get_program():
    if "nc" not in _CACHE:
        _CACHE["nc"] = _build_program()
    return _CACHE["nc"]


def make_core_inputs(x, w_attn, b_attn, w_proj, core):
    """Host-side shard preparation for one core."""
    b = core // 4
    g = core % 4
    heads = [4 * g + i for i in range(HPC)]

    xT = np.ascontiguousarray(np.asarray(x[b], np.float32).T)

    def qcols(h):
        return w_attn[:, h * DH : (h + 1) * DH]

    def kcols(h):
        return w_attn[:, C + h * DH : C + (h + 1) * DH]

    def vcols(h):
        return w_attn[:, 2 * C + h * DH : 2 * C + (h + 1) * DH]

    h0, h1, h2, h3 = heads
    wqk = np.ascontiguousarray(
        np.concatenate(
            [qcols(h0), qcols(h1), kcols(h0), kcols(h1),
             qcols(h2), qcols(h3), kcols(h2), kcols(h3)],
            axis=1,
        ).astype(np.float32)
    )
    wv = np.ascontiguousarray(
        np.concatenate([vcols(h) for h in heads], axis=1).astype(np.float32)
    )
    bqk = np.stack(
        [
            np.concatenate([b_attn[h0 * DH : (h0 + 1) * DH], b_attn[h1 * DH : (h1 + 1) * DH]]),
            np.concatenate([b_attn[C + h0 * DH : C + (h0 + 1) * DH], b_attn[C + h1 * DH : C + (h1 + 1) * DH]]),
            np.concatenate([b_attn[h2 * DH : (h2 + 1) * DH], b_attn[h3 * DH : (h3 + 1) * DH]]),
            np.concatenate([b_attn[C + h2 * DH : C + (h2 + 1) * DH], b_attn[C + h3 * DH : C + (h3 + 1) * DH]]),
        ]
    ).astype(np.float32)
    bv = np.concatenate(
        [b_attn[2 * C + h * DH : 2 * C + (h + 1) * DH] for h in heads]
    ).astype(np.float32)
    wo = np.ascontiguousarray(
        w_proj[heads[0] * DH : (heads[-1] + 1) * DH, :].astype(np.float32)
    )
    import ml_dtypes

    mask = np.triu(np.ones((P, P))).astype(ml_dtypes.bfloat16)
    return {
        "xT": xT,
        "wqk": wqk,
        "wv": wv,
        "wo": wo,
        "bqk": np.ascontiguousarray(bqk),
        "bv": np.ascontiguousarray(bv),
        "mask": mask,
    }


def kernel(x, w_attn, b_attn, w_proj, b_proj):
    from concourse.bass_utils import run_bass_kernel_spmd

    x = np.asarray(x, np.float32)
    w_attn = np.asarray(w_attn, np.float32)
    b_attn = np.asarray(b_attn, np.float32)
    w_proj = np.asarray(w_proj, np.float32)
    b_proj = np.asarray(b_proj, np.float32)

    nc = get_program()
    in_maps = [
        make_core_inputs(x, w_attn, b_attn, w_proj, core) for core in range(NCORES)
    ]
    res = run_bass_kernel_spmd(nc, in_maps, core_ids=list(range(NCORES)))
    outs = [m["out"] for m in res.results]

    y = np.empty((B, T, C), np.float32)
    for b in range(B):
        y[b] = outs[4 * b] + outs[4 * b + 1] + outs[4 * b + 2] + outs[4 * b + 3]
        y[b] += b_proj[None, :]
    return y
